# revision 35
# baseline (speedup 1.0000x reference)
"""LATTE GNN forward on 8 Trainium2 NeuronCores (v2, bf16 pipeline).

Math collapse (see reference): per-edge message is v[dst], softmax weights
sum to 1, so edge aggregation = v * mask. Masks are binary, so all mask=1
relations share the logit lrelu(vl+vr) and the relation-softmax collapses:
    v       = feat @ Wr                      (bias zero on fast path)
    vl,vr,vs= per-(n,h) projections of v     (extra matmul columns)
    eA      = exp(lrelu(vl+vr));  denA = sum_h eA
    s4[n,h] = eA * cnt[n]/denA    (cnt = 1 + #relations with an in-edge)
    mu      = sum_h s4*vs / 256;  mc = mu/s4
    w       = v - mc              (o - mu = s4*w exactly)
    var     = sum_h s4^2 * sum_c w^2 / 256
    rstd    = exp(-0.5*ln(var+eps))
    y       = max(w,0) * (s4*rstd)           (gamma=1, beta=0 fast path)
v is stored (c,h)-major (d' = c*4+h) so per-(tile,h) broadcast operands are
innermost-stride-1 -> bf16 2x DVE mode. Host un-permutes columns at the end.
Node-sharded 8 cores x 6250 rows (padded 6272 = 49*128).
"""

import numpy as np
import ml_dtypes

N, D, H, C, M = 50000, 256, 4, 64, 3
NCORES = 8
RPC = N // NCORES          # 6250
NT = 49
RPAD = NT * 128            # 6272
EPS = 1e-5
BF = ml_dtypes.bfloat16

# pipeline structure
GROUPS = [(0, 14), (14, 28), (28, 42), (42, 49)]   # tile ranges (small tail)
CH = 2                                             # tiles per PSUM chunk
NCHUNK = 25                                        # 24*2 + 1
CHUNK_BUFS = 4                                     # PSUM: 4 chunk banks + 4 smalls
FT_PIECES = [(0, 2), (2, 5), (5, 9), (9, 17), (17, 25), (25, 37), (37, 49)]

_CACHE = {}
LAST_RESULT = None


def _build(fast=True):
    import concourse.bass as bass
    import concourse.mybir as mybir
    from concourse.tile import TileContext

    fp32 = mybir.dt.float32
    bf16 = mybir.dt.bfloat16
    AF = mybir.ActivationFunctionType
    OP = mybir.AluOpType

    nc = bass.Bass()
    ftd = nc.declare_dram_parameter("ftT", [128, 2, RPAD], bf16, isOutput=False)
    wcd = nc.declare_dram_parameter("wcst", [128, 2, 268], bf16, isOutput=False)
    cntd = nc.declare_dram_parameter("cnt", [128, NT], fp32, isOutput=False)
    outd = nc.declare_dram_parameter("out", [128, NT * 256], bf16, isOutput=True)
    if not fast:
        # gb[:,0,0:256]=gamma (c,h)-major, gb[:,1,0:256]=beta,
        # gb[:,2,:]=bias row [br' | abias] (c,h)-major, applied via 1-row matmul
        gbd = nc.declare_dram_parameter("gb", [128, 3, 268], fp32, isOutput=False)

    with TileContext(nc) as tc:
        with (
            tc.tile_pool(name="const", bufs=1) as cpool,
            tc.tile_pool(name="sb", bufs=1) as sbp,
            tc.tile_pool(name="sm", bufs=1) as smp,
            tc.tile_pool(name="pv", bufs=3, space="PSUM") as pvp,
            tc.tile_pool(name="psm", bufs=1, space="PSUM") as psp,
        ):
            # ---- act table warmup (one set: natural_log_exp_and_others) ----
            warm = cpool.tile([128, 8], fp32, tag="warm")
            nc.gpsimd.memset(warm[:], 0.0)
            warm2 = cpool.tile([128, 8], fp32, tag="warm2")
            nc.scalar.activation(warm2[:], warm[:], AF.Exp)
            nc.scalar.activation(warm2[:], warm2[:], AF.Ln)
            eps_sb = cpool.tile([128, 1], fp32, tag="eps")
            nc.gpsimd.memset(eps_sb[:], EPS)

            # ---- constants ----
            w_sb = cpool.tile([128, 2, 268], bf16, tag="wc")
            nc.gpsimd.dma_start(out=w_sb[:], in_=wcd[:])
            cnt_sb = cpool.tile([128, NT], fp32, tag="cnt")
            nc.gpsimd.dma_start(out=cnt_sb[:], in_=cntd[:])
            if not fast:
                gb_sb = cpool.tile([128, 3, 268], fp32, tag="gb")
                nc.gpsimd.dma_start(out=gb_sb[:], in_=gbd[:])
                ones_sb = cpool.tile([128, 128], bf16, tag="ones")
                nc.gpsimd.memset(ones_sb[:], 1.0)
                brow = cpool.tile([128, 268], bf16, tag="brow")
                nc.vector.tensor_copy(out=brow[:], in_=gb_sb[:, 2, :])

            # ---- feature tiles (stationary operands), staircase DMA pieces
            # alternating between two queues (sync / tensor-issued) ----
            ft = sbp.tile([128, 2, RPAD], bf16, tag="ft")
            for i, (a, b) in enumerate(FT_PIECES):
                eng = nc.sync if i % 2 == 0 else nc.gpsimd
                eng.dma_start(out=ft[:, :, a * 128:b * 128],
                              in_=ftd[:, :, a * 128:b * 128])

            # ---- big SBUF buffers ----
            v_all = sbp.tile([128, NT, 256], bf16, tag="v")
            w_all = sbp.tile([128, NT, 256], bf16, tag="w")
            q_all = sbp.tile([128, NT, 256], bf16, tag="q")    # v^2 / tree scratch
            y_all = sbp.tile([128, NT, 256], bf16, tag="y")

            # ---- small per-(t,h) tensors ----
            sm_all = smp.tile([128, NT, 12], fp32, tag="sml")  # vl|vr|vs
            lvr = smp.tile([128, NT, 4], fp32, tag="lvr")
            eA = smp.tile([128, NT, 4], fp32, tag="eA")
            den = smp.tile([128, NT], fp32, tag="den")
            rq = smp.tile([128, NT], fp32, tag="rq")
            s4 = smp.tile([128, NT, 4], fp32, tag="s4")
            mus = smp.tile([128, NT, 4], fp32, tag="mus")
            mean = smp.tile([128, NT], fp32, tag="mean")
            rs4 = smp.tile([128, NT, 4], fp32, tag="rs4")
            mcn = smp.tile([128, NT, 4], bf16, tag="mcn")
            s4sq = smp.tile([128, NT, 4], fp32, tag="s4sq")
            prod = smp.tile([128, NT, 4], fp32, tag="prod")
            o2 = smp.tile([128, NT], fp32, tag="o2")
            rstd = smp.tile([128, NT], fp32, tag="rstd")
            spp = smp.tile([128, NT, 4], bf16, tag="spp")

            # ---- PSUM ----
            # one smalls bank per group (PE-W and DVE-R of the same PSUM bank
            # must never overlap in time -> reader waits for the whole tile)
            smS = [psp.tile([128, (GROUPS[g][1] - GROUPS[g][0]) * 12], fp32,
                            tag=f"sm{g}", name=f"smS{g}") for g in range(4)]
            vch = [None] * NCHUNK

            def pe_chunk(c):
                for t in range(c * CH, min((c + 1) * CH, NT)):
                    sl = t % CH
                    if sl == 0 or vch[c] is None:
                        vch[c] = pvp.tile([128, CH * 256], fp32, tag="vch",
                                          name=f"vch{c}", bufs=CHUNK_BUFS)
                    vout = vch[c][:, sl * 256:(sl + 1) * 256]
                    g = next(i for i, (a, b) in enumerate(GROUPS) if a <= t < b)
                    smt = smS[g][:, (t - GROUPS[g][0]) * 12:
                                 (t - GROUPS[g][0] + 1) * 12]
                    f0 = ft[:, 0, t * 128:(t + 1) * 128]
                    f1 = ft[:, 1, t * 128:(t + 1) * 128]
                    last = fast  # general path appends bias matmuls
                    nc.tensor.matmul(vout, f0, w_sb[:, 0, 0:256],
                                     start=True, stop=False)
                    nc.tensor.matmul(smt, f0, w_sb[:, 0, 256:268],
                                     start=True, stop=False)
                    nc.tensor.matmul(vout, f1, w_sb[:, 1, 0:256],
                                     start=False, stop=last)
                    nc.tensor.matmul(smt, f1, w_sb[:, 1, 256:268],
                                     start=False, stop=last)
                    if not fast:
                        nc.tensor.matmul(vout, ones_sb[0:1, :],
                                         brow[0:1, 0:256],
                                         start=False, stop=True)
                        nc.tensor.matmul(smt, ones_sb[0:1, :],
                                         brow[0:1, 256:268],
                                         start=False, stop=True)

            def evac(c):
                # alternate engines: PSUM->SBUF copies run at 1x on both, so
                # splitting halves the per-engine cost; then square on Act
                # (q = v^2, consumed by the early variance tree)
                t1c = c * CH + (1 if (c == NCHUNK - 1 and NT % CH) else CH)
                src = vch[c][:, 0:(t1c - c * CH) * 256]
                dst = v_all[:, c * CH:t1c, :]
                nc.scalar.copy(
                    out=dst, in_=src.rearrange("p (t d) -> p t d", d=256))
                nc.scalar.activation(q_all[:, c * CH:t1c, :], dst, AF.Square)

            # ============ grouped, software-pipelined back half ============
            # Emission is in DATAFLOW order (tile framework derives deps from
            # program order); pipelining comes from interleaving group stages:
            # S1(0) S1(1) S2(0) S1(2) S2(1) S1(3) S2(2) S2(3).

            def stage1(g):
                t0, t1 = GROUPS[g]
                s = slice(t0, t1)
                nt = t1 - t0
                # -- smalls PSUM -> SBUF (whole-bank read: PE done with it) --
                nc.vector.tensor_copy(
                    out=sm_all[:, s, :],
                    in_=smS[g][:].rearrange("p (t c) -> p t c", c=12))
                # -- s-chain --
                nc.vector.tensor_tensor(out=lvr[:, s, :], in0=sm_all[:, s, 0:4],
                                        in1=sm_all[:, s, 4:8], op=OP.add)
                nc.vector.scalar_tensor_tensor(
                    out=lvr[:, s, :], in0=lvr[:, s, :], scalar=0.2,
                    in1=lvr[:, s, :], op0=OP.mult, op1=OP.max)
                nc.scalar.activation(eA[:, s, :], lvr[:, s, :], AF.Exp)
                nc.vector.tensor_reduce(out=den[:, s], in_=eA[:, s, :],
                                        axis=mybir.AxisListType.X, op=OP.add)
                nc.scalar.activation(den[:, s], den[:, s], AF.Ln)
                nc.scalar.activation(rq[:, s], den[:, s], AF.Exp, scale=-1.0)
                nc.vector.tensor_tensor(out=rq[:, s], in0=cnt_sb[:, s],
                                        in1=rq[:, s], op=OP.mult)
                nc.vector.tensor_tensor(
                    out=s4[:, s, :], in0=eA[:, s, :],
                    in1=rq[:, s].unsqueeze(2).broadcast_to((128, nt, 4)),
                    op=OP.mult)
                nc.vector.tensor_tensor(out=mus[:, s, :], in0=s4[:, s, :],
                                        in1=sm_all[:, s, 8:12], op=OP.mult)
                nc.vector.tensor_reduce(out=mean[:, s], in_=mus[:, s, :],
                                        axis=mybir.AxisListType.X, op=OP.add)
                nc.scalar.mul(mean[:, s], mean[:, s], 1.0 / 256.0)
                nc.scalar.activation(rs4[:, s, :], s4[:, s, :], AF.Ln)
                nc.scalar.activation(rs4[:, s, :], rs4[:, s, :], AF.Exp,
                                     scale=-1.0)
                # mcneg = -mean * (1/s4)
                nc.vector.scalar_tensor_tensor(
                    out=mcn[:, s, :], in0=rs4[:, s, :], scalar=-1.0,
                    in1=mean[:, s].unsqueeze(2).broadcast_to((128, nt, 4)),
                    op0=OP.mult, op1=OP.mult)

            def stage_tree(g):
                # add-tree over c of q = v^2 (in place, bf16 2x); h-major:
                # q[p, t, h, c] -> result lands at c=0 per (t,h)
                t0, t1 = GROUPS[g]
                q4 = q_all[:, t0:t1, :].rearrange("p t (h c) -> p t h c", h=4)
                cc = 64
                while cc > 1:
                    hh = cc // 2
                    nc.vector.tensor_tensor(out=q4[:, :, :, 0:hh],
                                            in0=q4[:, :, :, 0:hh],
                                            in1=q4[:, :, :, hh:cc], op=OP.add)
                    cc = hh

            def stage2(g):
                t0, t1 = GROUPS[g]
                s = slice(t0, t1)
                nt = t1 - t0
                # h-major merged views: [p, (t h), c]; per-(t,h) operands are
                # 3D step-1 + stride-0 -> legal for scalar_tensor_tensor
                v3 = v_all[:, t0:t1, :].rearrange("p t (h c) -> p (t h) c", h=4)
                w3 = w_all[:, t0:t1, :].rearrange("p t (h c) -> p (t h) c", h=4)
                y3 = y_all[:, t0:t1, :].rearrange("p t (h c) -> p (t h) c", h=4)
                mc3 = mcn[:, s, :].rearrange("p t h -> p (t h)").unsqueeze(
                    2).broadcast_to((128, nt * 4, 64))
                # -- B2: w = v + mcneg_bcast --
                nc.vector.tensor_tensor(out=w3, in0=v3, in1=mc3, op=OP.add)
                # -- sum_c w^2 = vsq4 + 2*mcn*vs + 64*mcn^2  (per t,h) --
                vsq4 = q_all[:, t0:t1, :].rearrange(
                    "p t (h c) -> p t h c", h=4)[:, :, :, 0:1].squeeze(3)
                nc.vector.tensor_tensor(out=prod[:, s, :], in0=mcn[:, s, :],
                                        in1=sm_all[:, s, 8:12], op=OP.mult)
                nc.scalar.activation(s4sq[:, s, :], mcn[:, s, :], AF.Square,
                                     accum_out=None)
                nc.vector.scalar_tensor_tensor(
                    out=prod[:, s, :], in0=prod[:, s, :], scalar=2.0,
                    in1=vsq4, op0=OP.mult, op1=OP.add)
                nc.vector.scalar_tensor_tensor(
                    out=prod[:, s, :], in0=s4sq[:, s, :], scalar=64.0,
                    in1=prod[:, s, :], op0=OP.mult, op1=OP.add)
                # -- o2 = sum_h s4^2 * wsq4 ; rstd = exp(-.5 ln(o2/256+eps)) --
                nc.scalar.activation(s4sq[:, s, :], s4[:, s, :], AF.Square)
                nc.vector.tensor_tensor(out=prod[:, s, :], in0=s4sq[:, s, :],
                                        in1=prod[:, s, :], op=OP.mult)
                nc.vector.tensor_reduce(out=o2[:, s], in_=prod[:, s, :],
                                        axis=mybir.AxisListType.X, op=OP.add)
                nc.scalar.activation(rstd[:, s], o2[:, s], AF.Ln,
                                     scale=1.0 / 256.0, bias=eps_sb[:])
                nc.scalar.activation(rstd[:, s], rstd[:, s], AF.Exp, scale=-0.5)
                # -- s'' = s4*rstd ; y = max(w,0)*s''_bcast (one fused op) --
                nc.vector.scalar_tensor_tensor(
                    out=spp[:, s, :], in0=s4[:, s, :], scalar=1.0,
                    in1=rstd[:, s].unsqueeze(2).broadcast_to((128, nt, 4)),
                    op0=OP.bypass, op1=OP.mult)
                sp3 = spp[:, s, :].rearrange("p t h -> p (t h)").unsqueeze(
                    2).broadcast_to((128, nt * 4, 64))
                if fast:
                    nc.vector.scalar_tensor_tensor(
                        out=y3, in0=w3, scalar=0.0, in1=sp3,
                        op0=OP.max, op1=OP.mult)
                else:
                    nc.vector.scalar_tensor_tensor(
                        out=y3, in0=w3, scalar=1.0, in1=sp3,
                        op0=OP.bypass, op1=OP.mult)
                    zf = y_all[:, t0:t1, :]
                    nc.vector.tensor_tensor(
                        out=zf, in0=zf,
                        in1=gb_sb[:, 0, :].unsqueeze(1).broadcast_to(
                            (128, nt, 256)), op=OP.mult)
                    nc.vector.tensor_tensor(
                        out=zf, in0=zf,
                        in1=gb_sb[:, 1, :].unsqueeze(1).broadcast_to(
                            (128, nt, 256)), op=OP.add)
                    nc.vector.tensor_scalar_max(zf, zf, 0.0)
                nc.sync.dma_start(out=outd[:, t0 * 256:t1 * 256],
                                  in_=y_all[:, t0:t1, :])

            # PE/evac interleaved so chunk slots are reused only after their
            # reader is emitted (program order = dependency order); stages
            # injected as soon as their group's chunks are evacuated.
            # group -> last chunk: g0: c6, g1: c13, g2: c20, g3: c24
            inject = {10: [lambda: stage1(0), lambda: stage_tree(0)],
                      17: [lambda: stage2(0), lambda: stage1(1),
                           lambda: stage_tree(1)],
                      24: [lambda: stage2(1), lambda: stage1(2),
                           lambda: stage_tree(2)]}
            for c in range(NCHUNK):
                if c >= CHUNK_BUFS:
                    evac(c - CHUNK_BUFS)
                pe_chunk(c)
                for fn in inject.get(c, []):
                    fn()
            for c in range(NCHUNK - CHUNK_BUFS, NCHUNK):
                evac(c)
            stage1(3)
            stage_tree(3)
            stage2(2)
            stage2(3)
    return nc


def _split_waits(bir_bytes):
    """Walrus on this stack only accepts one sync-wait per instruction.
    Split extra waits into standalone single-wait NoOps on the same
    engine queue (exact raw-bass semantics: in-order queue stalls)."""
    import orjson
    m = orjson.loads(bir_bytes)
    counter = [0]

    def proc(obj):
        if isinstance(obj, dict):
            for k, v in obj.items():
                if k == "instructions" and isinstance(v, list):
                    new = []
                    for ins in v:
                        si = ins.get("sync_info")
                        waits = (si or {}).get("on_wait") or []
                        lim = 0 if ins.get("opcode") == "ISA" else 1
                        if si and len(waits) > lim:
                            keep = waits[-lim:] if lim else []
                            for w in (waits[:-1] if lim else waits):
                                counter[0] += 1
                                new.append({
                                    "name": f"I-wsplit-{counter[0]}",
                                    "opcode": "EventSemaphore",
                                    "engine": ins.get("engine"),
                                    "ins": [], "outs": [],
                                    "debug": ins.get("debug"),
                                    "sync_info": {"on_update": [],
                                                  "on_wait": [w]},
                                })
                            si["on_wait"] = keep
                        new.append(ins)
                        proc(ins)
                    obj[k] = new
                else:
                    proc(v)
        elif isinstance(obj, list):
            for x in obj:
                proc(x)

    proc(m)
    return orjson.dumps(m)


def kernel(**inputs):
    global LAST_RESULT
    import os
    from concourse.bass_utils import run_bass_kernel_spmd

    feat = np.ascontiguousarray(np.asarray(inputs["feat"], dtype=np.float32))
    Wr = np.asarray(inputs["Wr"], dtype=np.float32)
    br = np.asarray(inputs["br"], dtype=np.float32)
    rl = np.asarray(inputs["rel_attn_l"], dtype=np.float32)
    rr = np.asarray(inputs["rel_attn_r"], dtype=np.float32)
    g = np.asarray(inputs["ln_gamma"], dtype=np.float32)
    b = np.asarray(inputs["ln_beta"], dtype=np.float32)

    fast = (not br.any()) and (not b.any()) and np.all(g == 1.0)

    # cnt[n] = 1 + #relations with >=1 incoming edge at n
    cnt = np.ones(N, np.float32)
    for m in range(M):
        dst = np.asarray(inputs[f"dst{m}"])
        cnt += (np.bincount(dst, minlength=N) > 0)

    # weight prep: standard h-major columns + smalls columns [vl|vr|vs]
    Wr3 = Wr.reshape(256, H, C)
    AL = np.einsum('khc,hc->kh', Wr3, rl)                # [256,4]
    AR = np.einsum('khc,hc->kh', Wr3, rr)
    AS = Wr3.sum(2)                                      # [256,4]
    Wfull = np.concatenate([Wr, AL, AR, AS], axis=1)     # [256, 268]
    wcst = np.ascontiguousarray(Wfull.reshape(2, 128, 268).transpose(1, 0, 2)
                                ).astype(BF)             # [128, 2, 268]

    key = ("v2", fast)
    if key not in _CACHE:
        nc0 = _build(fast=fast)
        _orig = nc0.to_json_bytes
        nc0.to_json_bytes = lambda: _split_waits(_orig())
        _CACHE[key] = nc0
    nc = _CACHE[key]

    in_maps = []
    for s in range(NCORES):
        fs = np.zeros((RPAD, 256), np.float32)
        fs[:RPC] = feat[s * RPC:(s + 1) * RPC]
        ftT = np.ascontiguousarray(
            fs.T.reshape(2, 128, RPAD).transpose(1, 0, 2)).astype(BF)
        cs = np.full(RPAD, 4.0, np.float32)
        cs[:RPC] = cnt[s * RPC:(s + 1) * RPC]
        cnt_pt = np.ascontiguousarray(cs.reshape(NT, 128).T)  # [128, NT]
        im = {"ftT": ftT, "wcst": wcst, "cnt": cnt_pt}
        if not fast:
            br3 = br.reshape(H, C)
            abias = np.concatenate([(br3 * rl).sum(1), (br3 * rr).sum(1),
                                    br3.sum(1)])             # [12]
            gb = np.zeros((128, 3, 268), np.float32)
            gb[:, 0, 0:256] = g
            gb[:, 1, 0:256] = b
            gb[:, 2, 0:256] = br
            gb[:, 2, 256:268] = abias
            im["gb"] = gb
        in_maps.append(im)

    trace = bool(int(os.environ.get("KERNEL_TRACE", "0")))
    res = run_bass_kernel_spmd(nc, in_maps, list(range(NCORES)), trace=trace)
    LAST_RESULT = res

    outs = []
    for s in range(NCORES):
        y = np.asarray(res.results[s]["out"]).astype(np.float32)
        y = y.reshape(128, NT, 256).transpose(1, 0, 2).reshape(RPAD, 256)[:RPC]
        outs.append(y)
    return np.concatenate(outs, axis=0)


# revision 39
# speedup vs baseline: 1.0245x; 1.0245x over previous
"""LATTE GNN forward on 8 Trainium2 NeuronCores (v2, bf16 pipeline).

Math collapse (see reference): per-edge message is v[dst], softmax weights
sum to 1, so edge aggregation = v * mask. Masks are binary, so all mask=1
relations share the logit lrelu(vl+vr) and the relation-softmax collapses:
    v       = feat @ Wr                      (bias zero on fast path)
    vl,vr,vs= per-(n,h) projections of v     (extra matmul columns)
    eA      = exp(lrelu(vl+vr));  denA = sum_h eA
    s4[n,h] = eA * cnt[n]/denA    (cnt = 1 + #relations with an in-edge)
    mu      = sum_h s4*vs / 256;  mc = mu/s4
    w       = v - mc              (o - mu = s4*w exactly)
    var     = sum_h s4^2 * sum_c w^2 / 256
    rstd    = exp(-0.5*ln(var+eps))
    y       = max(w,0) * (s4*rstd)           (gamma=1, beta=0 fast path)
v is stored (c,h)-major (d' = c*4+h) so per-(tile,h) broadcast operands are
innermost-stride-1 -> bf16 2x DVE mode. Host un-permutes columns at the end.
Node-sharded 8 cores x 6250 rows (padded 6272 = 49*128).
"""

import numpy as np
import ml_dtypes

N, D, H, C, M = 50000, 256, 4, 64, 3
NCORES = 8
RPC = N // NCORES          # 6250
NT = 49
RPAD = NT * 128            # 6272
EPS = 1e-5
BF = ml_dtypes.bfloat16

# pipeline structure
GROUPS = [(0, 14), (14, 28), (28, 42), (42, 49)]   # tile ranges (small tail)
CH = 2                                             # tiles per PSUM chunk
NCHUNK = 25                                        # 24*2 + 1
CHUNK_BUFS = 4                                     # PSUM: 4 chunk banks + 4 smalls
FT_PIECES = [(0, 2), (2, 5), (5, 9), (9, 17), (17, 25), (25, 37), (37, 49)]

_CACHE = {}
LAST_RESULT = None


def _build(fast=True):
    import concourse.bass as bass
    import concourse.mybir as mybir
    from concourse.tile import TileContext

    fp32 = mybir.dt.float32
    bf16 = mybir.dt.bfloat16
    AF = mybir.ActivationFunctionType
    OP = mybir.AluOpType

    nc = bass.Bass()
    ftd = nc.declare_dram_parameter("ftT", [128, 2, RPAD], bf16, isOutput=False)
    wcd = nc.declare_dram_parameter("wcst", [128, 2, 268], bf16, isOutput=False)
    cntd = nc.declare_dram_parameter("cnt", [128, NT], fp32, isOutput=False)
    outd = nc.declare_dram_parameter("out", [128, NT * 256], bf16, isOutput=True)
    if not fast:
        # gb[:,0,0:256]=gamma (c,h)-major, gb[:,1,0:256]=beta,
        # gb[:,2,:]=bias row [br' | abias] (c,h)-major, applied via 1-row matmul
        gbd = nc.declare_dram_parameter("gb", [128, 3, 268], fp32, isOutput=False)

    with TileContext(nc) as tc:
        with (
            tc.tile_pool(name="const", bufs=1) as cpool,
            tc.tile_pool(name="sb", bufs=1) as sbp,
            tc.tile_pool(name="sm", bufs=1) as smp,
            tc.tile_pool(name="pv", bufs=3, space="PSUM") as pvp,
            tc.tile_pool(name="psm", bufs=1, space="PSUM") as psp,
        ):
            # ---- act table warmup (one set: natural_log_exp_and_others) ----
            warm = cpool.tile([128, 8], fp32, tag="warm")
            nc.gpsimd.memset(warm[:], 0.0)
            warm2 = cpool.tile([128, 8], fp32, tag="warm2")
            nc.scalar.activation(warm2[:], warm[:], AF.Exp)
            nc.scalar.activation(warm2[:], warm2[:], AF.Ln)
            eps_sb = cpool.tile([128, 1], fp32, tag="eps")
            nc.gpsimd.memset(eps_sb[:], EPS)

            # ---- constants ----
            w_sb = cpool.tile([128, 2, 268], bf16, tag="wc")
            nc.gpsimd.dma_start(out=w_sb[:], in_=wcd[:])
            cnt_sb = cpool.tile([128, NT], fp32, tag="cnt")
            nc.gpsimd.dma_start(out=cnt_sb[:], in_=cntd[:])
            if not fast:
                gb_sb = cpool.tile([128, 3, 268], fp32, tag="gb")
                nc.gpsimd.dma_start(out=gb_sb[:], in_=gbd[:])
                ones_sb = cpool.tile([128, 128], bf16, tag="ones")
                nc.gpsimd.memset(ones_sb[:], 1.0)
                brow = cpool.tile([128, 268], bf16, tag="brow")
                nc.vector.tensor_copy(out=brow[:], in_=gb_sb[:, 2, :])

            # ---- feature tiles (stationary operands), staircase DMA pieces
            # alternating between two queues (sync / tensor-issued) ----
            ft = sbp.tile([128, 2, RPAD], bf16, tag="ft")
            for i, (a, b) in enumerate(FT_PIECES):
                eng = nc.sync if i % 2 == 0 else nc.gpsimd
                eng.dma_start(out=ft[:, :, a * 128:b * 128],
                              in_=ftd[:, :, a * 128:b * 128])

            # ---- big SBUF buffers ----
            v_all = sbp.tile([128, NT, 256], bf16, tag="v")
            w_all = sbp.tile([128, NT, 256], bf16, tag="w")
            q_all = sbp.tile([128, NT, 256], bf16, tag="q")    # v^2 / tree scratch
            y_all = sbp.tile([128, NT, 256], bf16, tag="y")

            # ---- small per-(t,h) tensors ----
            sm_all = smp.tile([128, NT, 12], fp32, tag="sml")  # vl|vr|vs
            lvr = smp.tile([128, NT, 4], fp32, tag="lvr")
            eA = smp.tile([128, NT, 4], fp32, tag="eA")
            den = smp.tile([128, NT], fp32, tag="den")
            rq = smp.tile([128, NT], fp32, tag="rq")
            s4 = smp.tile([128, NT, 4], fp32, tag="s4")
            mus = smp.tile([128, NT, 4], fp32, tag="mus")
            mean = smp.tile([128, NT], fp32, tag="mean")
            rs4 = smp.tile([128, NT, 4], fp32, tag="rs4")
            mcn = smp.tile([128, NT, 4], bf16, tag="mcn")
            s4sq = smp.tile([128, NT, 4], fp32, tag="s4sq")
            prod = smp.tile([128, NT, 4], fp32, tag="prod")
            o2 = smp.tile([128, NT], fp32, tag="o2")
            rstd = smp.tile([128, NT], fp32, tag="rstd")
            spp = smp.tile([128, NT, 4], bf16, tag="spp")

            # ---- PSUM ----
            # one smalls bank per group (PE-W and DVE-R of the same PSUM bank
            # must never overlap in time -> reader waits for the whole tile)
            smS = [psp.tile([128, (GROUPS[g][1] - GROUPS[g][0]) * 12], fp32,
                            tag=f"sm{g}", name=f"smS{g}") for g in range(4)]
            vch = [None] * NCHUNK

            def pe_chunk(c):
                for t in range(c * CH, min((c + 1) * CH, NT)):
                    sl = t % CH
                    if sl == 0 or vch[c] is None:
                        vch[c] = pvp.tile([128, CH * 256], fp32, tag="vch",
                                          name=f"vch{c}", bufs=CHUNK_BUFS)
                    vout = vch[c][:, sl * 256:(sl + 1) * 256]
                    g = next(i for i, (a, b) in enumerate(GROUPS) if a <= t < b)
                    smt = smS[g][:, (t - GROUPS[g][0]) * 12:
                                 (t - GROUPS[g][0] + 1) * 12]
                    f0 = ft[:, 0, t * 128:(t + 1) * 128]
                    f1 = ft[:, 1, t * 128:(t + 1) * 128]
                    last = fast  # general path appends bias matmuls
                    nc.tensor.matmul(vout, f0, w_sb[:, 0, 0:256],
                                     start=True, stop=False)
                    nc.tensor.matmul(smt, f0, w_sb[:, 0, 256:268],
                                     start=True, stop=False)
                    nc.tensor.matmul(vout, f1, w_sb[:, 1, 0:256],
                                     start=False, stop=last)
                    nc.tensor.matmul(smt, f1, w_sb[:, 1, 256:268],
                                     start=False, stop=last)
                    if not fast:
                        nc.tensor.matmul(vout, ones_sb[0:1, :],
                                         brow[0:1, 0:256],
                                         start=False, stop=True)
                        nc.tensor.matmul(smt, ones_sb[0:1, :],
                                         brow[0:1, 256:268],
                                         start=False, stop=True)

            def evac(c):
                # alternate engines: PSUM->SBUF copies run at 1x on both, so
                # splitting halves the per-engine cost; then square on Act
                # (q = v^2, consumed by the early variance tree)
                t1c = c * CH + (1 if (c == NCHUNK - 1 and NT % CH) else CH)
                src = vch[c][:, 0:(t1c - c * CH) * 256]
                dst = v_all[:, c * CH:t1c, :]
                nc.scalar.copy(
                    out=dst, in_=src.rearrange("p (t d) -> p t d", d=256))

            def stage_sq(g):
                # q = v^2 on Act, batched per group (reads SBUF: may lag PE)
                t0, t1 = GROUPS[g]
                nc.scalar.activation(q_all[:, t0:t1, :], v_all[:, t0:t1, :],
                                     AF.Square)

            # ============ grouped, software-pipelined back half ============
            # Emission is in DATAFLOW order (tile framework derives deps from
            # program order); pipelining comes from interleaving group stages:
            # S1(0) S1(1) S2(0) S1(2) S2(1) S1(3) S2(2) S2(3).

            def stage1(g):
                t0, t1 = GROUPS[g]
                s = slice(t0, t1)
                nt = t1 - t0
                # -- smalls PSUM -> SBUF (whole-bank read: PE done with it) --
                nc.vector.tensor_copy(
                    out=sm_all[:, s, :],
                    in_=smS[g][:].rearrange("p (t c) -> p t c", c=12))
                # -- s-chain (only exp touches the Act engine) --
                nc.vector.tensor_tensor(out=lvr[:, s, :], in0=sm_all[:, s, 0:4],
                                        in1=sm_all[:, s, 4:8], op=OP.add)
                nc.vector.scalar_tensor_tensor(
                    out=lvr[:, s, :], in0=lvr[:, s, :], scalar=0.2,
                    in1=lvr[:, s, :], op0=OP.mult, op1=OP.max)
                nc.scalar.activation(eA[:, s, :], lvr[:, s, :], AF.Exp)
                nc.vector.tensor_reduce(out=den[:, s], in_=eA[:, s, :],
                                        axis=mybir.AxisListType.X, op=OP.add)
                nc.vector.reciprocal(rq[:, s], den[:, s])
                nc.vector.tensor_tensor(out=rq[:, s], in0=cnt_sb[:, s],
                                        in1=rq[:, s], op=OP.mult)
                nc.vector.tensor_tensor(
                    out=s4[:, s, :], in0=eA[:, s, :],
                    in1=rq[:, s].unsqueeze(2).broadcast_to((128, nt, 4)),
                    op=OP.mult)
                nc.vector.tensor_tensor(out=mus[:, s, :], in0=s4[:, s, :],
                                        in1=sm_all[:, s, 8:12], op=OP.mult)
                nc.vector.tensor_reduce(out=mean[:, s], in_=mus[:, s, :],
                                        axis=mybir.AxisListType.X, op=OP.add)
                nc.vector.reciprocal(rs4[:, s, :], s4[:, s, :])
                # mcneg = -(mean/256) * (1/s4)
                nc.vector.scalar_tensor_tensor(
                    out=mcn[:, s, :], in0=rs4[:, s, :], scalar=-1.0 / 256.0,
                    in1=mean[:, s].unsqueeze(2).broadcast_to((128, nt, 4)),
                    op0=OP.mult, op1=OP.mult)

            def stage_tree(g):
                # add-tree over c of q = v^2 (in place, bf16 2x); h-major:
                # q[p, t, h, c] -> result lands at c=0 per (t,h)
                t0, t1 = GROUPS[g]
                q4 = q_all[:, t0:t1, :].rearrange("p t (h c) -> p t h c", h=4)
                cc = 64
                while cc > 1:
                    hh = cc // 2
                    nc.vector.tensor_tensor(out=q4[:, :, :, 0:hh],
                                            in0=q4[:, :, :, 0:hh],
                                            in1=q4[:, :, :, hh:cc], op=OP.add)
                    cc = hh

            def stage2(g):
                t0, t1 = GROUPS[g]
                s = slice(t0, t1)
                nt = t1 - t0
                # h-major merged views: [p, (t h), c]; per-(t,h) operands are
                # 3D step-1 + stride-0 -> legal for scalar_tensor_tensor
                v3 = v_all[:, t0:t1, :].rearrange("p t (h c) -> p (t h) c", h=4)
                w3 = w_all[:, t0:t1, :].rearrange("p t (h c) -> p (t h) c", h=4)
                y3 = y_all[:, t0:t1, :].rearrange("p t (h c) -> p (t h) c", h=4)
                mc3 = mcn[:, s, :].rearrange("p t h -> p (t h)").unsqueeze(
                    2).broadcast_to((128, nt * 4, 64))
                # -- B2: w = v + mcneg_bcast --
                nc.vector.tensor_tensor(out=w3, in0=v3, in1=mc3, op=OP.add)
                # -- sum_c w^2 = vsq4 + 2*mcn*vs + 64*mcn^2  (per t,h) --
                vsq4 = q_all[:, t0:t1, :].rearrange(
                    "p t (h c) -> p t h c", h=4)[:, :, :, 0:1].squeeze(3)
                nc.vector.tensor_tensor(out=prod[:, s, :], in0=mcn[:, s, :],
                                        in1=sm_all[:, s, 8:12], op=OP.mult)
                nc.vector.scalar_tensor_tensor(
                    out=s4sq[:, s, :], in0=mcn[:, s, :], scalar=1.0,
                    in1=mcn[:, s, :], op0=OP.bypass, op1=OP.mult)
                nc.vector.scalar_tensor_tensor(
                    out=prod[:, s, :], in0=prod[:, s, :], scalar=2.0,
                    in1=vsq4, op0=OP.mult, op1=OP.add)
                nc.vector.scalar_tensor_tensor(
                    out=prod[:, s, :], in0=s4sq[:, s, :], scalar=64.0,
                    in1=prod[:, s, :], op0=OP.mult, op1=OP.add)
                # -- o2 = sum_h s4^2 * wsq4 ; rstd = exp(-.5 ln(o2/256+eps)) --
                nc.vector.scalar_tensor_tensor(
                    out=s4sq[:, s, :], in0=s4[:, s, :], scalar=1.0,
                    in1=s4[:, s, :], op0=OP.bypass, op1=OP.mult)
                nc.vector.tensor_tensor(out=prod[:, s, :], in0=s4sq[:, s, :],
                                        in1=prod[:, s, :], op=OP.mult)
                nc.vector.tensor_reduce(out=o2[:, s], in_=prod[:, s, :],
                                        axis=mybir.AxisListType.X, op=OP.add)
                nc.scalar.activation(rstd[:, s], o2[:, s], AF.Ln,
                                     scale=1.0 / 256.0, bias=eps_sb[:])
                nc.scalar.activation(rstd[:, s], rstd[:, s], AF.Exp, scale=-0.5)
                # -- s'' = s4*rstd ; y = max(w,0)*s''_bcast (one fused op) --
                nc.vector.scalar_tensor_tensor(
                    out=spp[:, s, :], in0=s4[:, s, :], scalar=1.0,
                    in1=rstd[:, s].unsqueeze(2).broadcast_to((128, nt, 4)),
                    op0=OP.bypass, op1=OP.mult)
                sp3 = spp[:, s, :].rearrange("p t h -> p (t h)").unsqueeze(
                    2).broadcast_to((128, nt * 4, 64))
                if fast:
                    nc.vector.scalar_tensor_tensor(
                        out=y3, in0=w3, scalar=0.0, in1=sp3,
                        op0=OP.max, op1=OP.mult)
                else:
                    nc.vector.scalar_tensor_tensor(
                        out=y3, in0=w3, scalar=1.0, in1=sp3,
                        op0=OP.bypass, op1=OP.mult)
                    zf = y_all[:, t0:t1, :]
                    nc.vector.tensor_tensor(
                        out=zf, in0=zf,
                        in1=gb_sb[:, 0, :].unsqueeze(1).broadcast_to(
                            (128, nt, 256)), op=OP.mult)
                    nc.vector.tensor_tensor(
                        out=zf, in0=zf,
                        in1=gb_sb[:, 1, :].unsqueeze(1).broadcast_to(
                            (128, nt, 256)), op=OP.add)
                    nc.vector.tensor_scalar_max(zf, zf, 0.0)
                nc.sync.dma_start(out=outd[:, t0 * 256:t1 * 256],
                                  in_=y_all[:, t0:t1, :])

            # PE/evac interleaved so chunk slots are reused only after their
            # reader is emitted (program order = dependency order); stages
            # injected as soon as their group's chunks are evacuated.
            # group -> last chunk: g0: c6, g1: c13, g2: c20, g3: c24
            inject = {10: [lambda: stage_sq(0), lambda: stage1(0),
                           lambda: stage_tree(0)],
                      17: [lambda: stage_sq(1), lambda: stage2(0),
                           lambda: stage1(1), lambda: stage_tree(1)],
                      24: [lambda: stage_sq(2), lambda: stage2(1),
                           lambda: stage1(2), lambda: stage_tree(2)]}
            for c in range(NCHUNK):
                if c >= CHUNK_BUFS:
                    evac(c - CHUNK_BUFS)
                pe_chunk(c)
                for fn in inject.get(c, []):
                    fn()
            for c in range(NCHUNK - CHUNK_BUFS, NCHUNK):
                evac(c)
            stage_sq(3)
            stage1(3)
            stage_tree(3)
            stage2(2)
            stage2(3)
    return nc


def _split_waits(bir_bytes):
    """Walrus on this stack only accepts one sync-wait per instruction.
    Split extra waits into standalone single-wait NoOps on the same
    engine queue (exact raw-bass semantics: in-order queue stalls)."""
    import orjson
    m = orjson.loads(bir_bytes)
    counter = [0]

    def proc(obj):
        if isinstance(obj, dict):
            for k, v in obj.items():
                if k == "instructions" and isinstance(v, list):
                    new = []
                    for ins in v:
                        si = ins.get("sync_info")
                        waits = (si or {}).get("on_wait") or []
                        lim = 0 if ins.get("opcode") == "ISA" else 1
                        if si and len(waits) > lim:
                            keep = waits[-lim:] if lim else []
                            for w in (waits[:-1] if lim else waits):
                                counter[0] += 1
                                new.append({
                                    "name": f"I-wsplit-{counter[0]}",
                                    "opcode": "EventSemaphore",
                                    "engine": ins.get("engine"),
                                    "ins": [], "outs": [],
                                    "debug": ins.get("debug"),
                                    "sync_info": {"on_update": [],
                                                  "on_wait": [w]},
                                })
                            si["on_wait"] = keep
                        new.append(ins)
                        proc(ins)
                    obj[k] = new
                else:
                    proc(v)
        elif isinstance(obj, list):
            for x in obj:
                proc(x)

    proc(m)
    return orjson.dumps(m)


def kernel(**inputs):
    global LAST_RESULT
    import os
    from concourse.bass_utils import run_bass_kernel_spmd

    feat = np.ascontiguousarray(np.asarray(inputs["feat"], dtype=np.float32))
    Wr = np.asarray(inputs["Wr"], dtype=np.float32)
    br = np.asarray(inputs["br"], dtype=np.float32)
    rl = np.asarray(inputs["rel_attn_l"], dtype=np.float32)
    rr = np.asarray(inputs["rel_attn_r"], dtype=np.float32)
    g = np.asarray(inputs["ln_gamma"], dtype=np.float32)
    b = np.asarray(inputs["ln_beta"], dtype=np.float32)

    fast = (not br.any()) and (not b.any()) and np.all(g == 1.0)

    # cnt[n] = 1 + #relations with >=1 incoming edge at n
    cnt = np.ones(N, np.float32)
    for m in range(M):
        dst = np.asarray(inputs[f"dst{m}"])
        cnt += (np.bincount(dst, minlength=N) > 0)

    # weight prep: standard h-major columns + smalls columns [vl|vr|vs]
    Wr3 = Wr.reshape(256, H, C)
    AL = np.einsum('khc,hc->kh', Wr3, rl)                # [256,4]
    AR = np.einsum('khc,hc->kh', Wr3, rr)
    AS = Wr3.sum(2)                                      # [256,4]
    Wfull = np.concatenate([Wr, AL, AR, AS], axis=1)     # [256, 268]
    wcst = np.ascontiguousarray(Wfull.reshape(2, 128, 268).transpose(1, 0, 2)
                                ).astype(BF)             # [128, 2, 268]

    key = ("v2", fast)
    if key not in _CACHE:
        nc0 = _build(fast=fast)
        _orig = nc0.to_json_bytes
        nc0.to_json_bytes = lambda: _split_waits(_orig())
        _CACHE[key] = nc0
    nc = _CACHE[key]

    in_maps = []
    for s in range(NCORES):
        fs = np.zeros((RPAD, 256), np.float32)
        fs[:RPC] = feat[s * RPC:(s + 1) * RPC]
        ftT = np.ascontiguousarray(
            fs.T.reshape(2, 128, RPAD).transpose(1, 0, 2)).astype(BF)
        cs = np.full(RPAD, 4.0, np.float32)
        cs[:RPC] = cnt[s * RPC:(s + 1) * RPC]
        cnt_pt = np.ascontiguousarray(cs.reshape(NT, 128).T)  # [128, NT]
        im = {"ftT": ftT, "wcst": wcst, "cnt": cnt_pt}
        if not fast:
            br3 = br.reshape(H, C)
            abias = np.concatenate([(br3 * rl).sum(1), (br3 * rr).sum(1),
                                    br3.sum(1)])             # [12]
            gb = np.zeros((128, 3, 268), np.float32)
            gb[:, 0, 0:256] = g
            gb[:, 1, 0:256] = b
            gb[:, 2, 0:256] = br
            gb[:, 2, 256:268] = abias
            im["gb"] = gb
        in_maps.append(im)

    trace = bool(int(os.environ.get("KERNEL_TRACE", "0")))
    res = run_bass_kernel_spmd(nc, in_maps, list(range(NCORES)), trace=trace)
    LAST_RESULT = res

    outs = []
    for s in range(NCORES):
        y = np.asarray(res.results[s]["out"]).astype(np.float32)
        y = y.reshape(128, NT, 256).transpose(1, 0, 2).reshape(RPAD, 256)[:RPC]
        outs.append(y)
    return np.concatenate(outs, axis=0)
